# revision 3
# baseline (speedup 1.0000x reference)
"""ANI-style per-species MLP (MoE hard routing) on 8 TRN2 NeuronCores.

Strategy:
  - Host: flatten atoms, sort by species, pad each species bucket to a
    multiple of 8*TILE, and deal equal per-species segments to each core.
    Every core therefore runs the SAME graph (SPMD) over its own atoms.
  - Device (per core): for each 512-atom tile of a species segment, run the
    4-layer MLP (384->160->128->96->1, CELU) with that species' weights only
    (4x less work than the dense reference). Matmuls in bf16 (fp32 PSUM
    accumulate), CELU via one ScalarE Exp pass + one fused custom DVE op:
        celu(v)+1 = relu(v) + min(exp(v), 1)       [v = z + b]
    The +1 shift is folded into the next layer's bias (b' = b - W^T 1).
  - Host: scatter per-atom energies back, add the L4 bias, per-molecule sum.
"""

import os
from contextlib import ExitStack

import numpy as np
import ml_dtypes

import concourse.bacc as bacc
import concourse.mybir as mybir
import concourse.tile as tile
from concourse.bass_utils import run_bass_kernel_spmd

BF16 = ml_dtypes.bfloat16
F32 = np.float32

N_CORES = 8
TILE = 512
D_AEV = 384
DH1, DH2, DH3 = 160, 128, 96
N_SPECIES = 4

_ACT = mybir.ActivationFunctionType


# --------------------------------------------------------------------------- #
# Fused CELU custom DVE op: out = min(in0, 1) + relu(in1 + s0)
#   in0 = exp(z + b) (SBUF f32, from ScalarE), in1 = z (PSUM f32), s0 = b.
# Result equals celu(z + b) + 1; the -1 is folded into the next layer's bias.
# --------------------------------------------------------------------------- #
def _register_celu_op():
    import concourse.dve_ops as dve_ops
    from concourse.dve_spec import Spec, Src0, Src1, C0, One, relu, minn, lower
    from concourse.dve_uop import DveOpSpec

    name = "CELU1_ANT"
    for op in dve_ops.OPS:
        if op.name == name:
            return op
    spec = Spec(
        body=minn(Src0, One) + relu(Src1 + C0),
        reference=lambda in0, in1, s0, s1, imm2: np.minimum(in0, 1.0)
        + np.maximum(in1 + s0, 0.0),
    )
    row = dve_ops._CUSTOM_DVE_ROW_BASE + len(dve_ops.OPS)
    assert row < 0x20, "custom DVE row field overflow"
    shas = {}
    for ver in ("v3", "v4"):
        d = DveOpSpec(name=name, opcode=row, uops=lower(spec, ver=ver), rd1_en=True)
        shas[ver] = d.sha(ver)
    op = dve_ops.DveOp(name, spec, False, shas)
    dve_ops.OPS.append(op)
    dve_ops.CUSTOM_DVE_SPECS[name] = spec
    dve_ops._SUB_OPCODE_FOR_NAME[name] = row
    return op


# --------------------------------------------------------------------------- #
# Graph builder (one core's SPMD program).
# seg_tiles: number of 512-atom tiles per species segment. ncore = 512*sum.
# --------------------------------------------------------------------------- #
def build_graph(seg_tiles):
    celu_op = _register_celu_op()
    dt = mybir.dt
    ncore = TILE * int(sum(seg_tiles))

    nc = bacc.Bacc("TRN2", target_bir_lowering=False, debug=False)

    x_ext = nc.dram_tensor("x", [128, 3, ncore], dt.bfloat16, kind="ExternalInput")
    w1_ext = nc.dram_tensor("w1", [128, 12 * DH1], dt.bfloat16, kind="ExternalInput")
    w2a_ext = nc.dram_tensor("w2a", [128, 4 * DH2], dt.bfloat16, kind="ExternalInput")
    w2b_ext = nc.dram_tensor("w2b", [32, 4 * DH2], dt.bfloat16, kind="ExternalInput")
    w3_ext = nc.dram_tensor("w3", [128, 4 * DH3], dt.bfloat16, kind="ExternalInput")
    w4_ext = nc.dram_tensor("w4", [96, 4], dt.bfloat16, kind="ExternalInput")
    b1a_ext = nc.dram_tensor("b1a", [128, 4], dt.float32, kind="ExternalInput")
    b1b_ext = nc.dram_tensor("b1b", [32, 4], dt.float32, kind="ExternalInput")
    b2_ext = nc.dram_tensor("b2", [128, 4], dt.float32, kind="ExternalInput")
    b3_ext = nc.dram_tensor("b3", [96, 4], dt.float32, kind="ExternalInput")
    out_ext = nc.dram_tensor("out", [1, ncore], dt.float32, kind="ExternalOutput")

    with tile.TileContext(nc) as tc, ExitStack() as ctx:
        wpool = ctx.enter_context(tc.tile_pool(name="w", bufs=1))
        xpool = ctx.enter_context(tc.tile_pool(name="x", bufs=4))
        epool = ctx.enter_context(tc.tile_pool(name="e", bufs=3))
        spool = ctx.enter_context(tc.tile_pool(name="s", bufs=3))
        p1a = ctx.enter_context(tc.tile_pool(name="p1a", bufs=2, space="PSUM"))
        p1b = ctx.enter_context(tc.tile_pool(name="p1b", bufs=1, space="PSUM"))
        p2 = ctx.enter_context(tc.tile_pool(name="p2", bufs=2, space="PSUM"))
        p3 = ctx.enter_context(tc.tile_pool(name="p3", bufs=1, space="PSUM"))
        p4 = ctx.enter_context(tc.tile_pool(name="p4", bufs=2, space="PSUM"))

        # --- load weights/biases once ---
        w1_sb = wpool.tile([128, 12 * DH1], dt.bfloat16)
        w2a_sb = wpool.tile([128, 4 * DH2], dt.bfloat16)
        w2b_sb = wpool.tile([32, 4 * DH2], dt.bfloat16)
        w3_sb = wpool.tile([128, 4 * DH3], dt.bfloat16)
        w4_sb = wpool.tile([96, 4], dt.bfloat16)
        b1a_sb = wpool.tile([128, 4], dt.float32)
        b1b_sb = wpool.tile([32, 4], dt.float32)
        b2_sb = wpool.tile([128, 4], dt.float32)
        b3_sb = wpool.tile([96, 4], dt.float32)
        for sb, ext in [
            (w1_sb, w1_ext), (w2a_sb, w2a_ext), (w2b_sb, w2b_ext),
            (w3_sb, w3_ext), (w4_sb, w4_ext), (b1a_sb, b1a_ext),
            (b1b_sb, b1b_ext), (b2_sb, b2_ext), (b3_sb, b3_ext),
        ]:
            nc.sync.dma_start(sb[:], ext[:])

        def emit_tile(s, g0):
            xt = xpool.tile([128, 3, TILE], dt.bfloat16)
            nc.sync.dma_start(xt[:], x_ext[:, :, g0 : g0 + TILE])

            # ---- L1: 384 -> 160 (main 128 + rem 32) ----
            z1a = p1a.tile([128, TILE], dt.float32)
            z1b = p1b.tile([32, TILE], dt.float32)
            for k in range(3):
                base = (s * 3 + k) * DH1
                nc.tensor.matmul(
                    z1a[:], w1_sb[:, base : base + 128], xt[:, k, :],
                    start=(k == 0), stop=(k == 2),
                )
            for k in range(3):
                base = (s * 3 + k) * DH1
                nc.tensor.matmul(
                    z1b[:], w1_sb[:, base + 128 : base + DH1], xt[:, k, :],
                    start=(k == 0), stop=(k == 2),
                )
            e1a = epool.tile([128, TILE], dt.float32, tag="e1a")
            nc.scalar.activation(e1a[:], z1a[:], _ACT.Exp, bias=b1a_sb[:, s : s + 1])
            s1a = spool.tile([128, TILE], dt.bfloat16, tag="s1a")
            nc.vector._custom_dve(
                celu_op, out=s1a[:], in0=e1a[:], in1=z1a[:], s0=b1a_sb[:, s : s + 1]
            )
            e1b = epool.tile([32, TILE], dt.float32, tag="e1b")
            nc.scalar.activation(e1b[:], z1b[:], _ACT.Exp, bias=b1b_sb[:, s : s + 1])
            s1b = spool.tile([32, TILE], dt.bfloat16, tag="s1b")
            nc.vector._custom_dve(
                celu_op, out=s1b[:], in0=e1b[:], in1=z1b[:], s0=b1b_sb[:, s : s + 1]
            )

            # ---- L2: 160 -> 128 ----
            z2 = p2.tile([128, TILE], dt.float32)
            nc.tensor.matmul(
                z2[:], w2a_sb[:, s * DH2 : (s + 1) * DH2], s1a[:],
                start=True, stop=False,
            )
            nc.tensor.matmul(
                z2[:], w2b_sb[:, s * DH2 : (s + 1) * DH2], s1b[:],
                start=False, stop=True,
            )
            e2 = epool.tile([128, TILE], dt.float32, tag="e2")
            nc.scalar.activation(e2[:], z2[:], _ACT.Exp, bias=b2_sb[:, s : s + 1])
            s2 = spool.tile([128, TILE], dt.bfloat16, tag="s2")
            nc.vector._custom_dve(
                celu_op, out=s2[:], in0=e2[:], in1=z2[:], s0=b2_sb[:, s : s + 1]
            )

            # ---- L3: 128 -> 96 ----
            z3 = p3.tile([96, TILE], dt.float32)
            nc.tensor.matmul(z3[:], w3_sb[:, s * DH3 : (s + 1) * DH3], s2[:])
            e3 = epool.tile([96, TILE], dt.float32, tag="e3")
            nc.scalar.activation(e3[:], z3[:], _ACT.Exp, bias=b3_sb[:, s : s + 1])
            s3 = spool.tile([96, TILE], dt.bfloat16, tag="s3")
            nc.vector._custom_dve(
                celu_op, out=s3[:], in0=e3[:], in1=z3[:], s0=b3_sb[:, s : s + 1]
            )

            # ---- L4: 96 -> 1 (bias added on host) ----
            z4 = p4.tile([1, TILE], dt.float32)
            nc.tensor.matmul(z4[:], w4_sb[:, s : s + 1], s3[:])
            en = spool.tile([1, TILE], dt.float32, tag="en")
            nc.scalar.copy(en[:], z4[:])
            nc.sync.dma_start(out_ext[:, g0 : g0 + TILE], en[:])

        g0 = 0
        for s in range(N_SPECIES):
            for _ in range(int(seg_tiles[s])):
                emit_tile(s, g0)
                g0 += TILE

    nc.compile()
    return nc


# --------------------------------------------------------------------------- #
# Host-side input prep / output unpack.
# --------------------------------------------------------------------------- #
def _prep_weights(W1, b1, W2, b2, W3, b3, W4, b4):
    # w1: [128, 12*DH1], column block (s*3+k) holds W1[s][128k:128k+128, :]
    w1 = np.empty((128, 12 * DH1), BF16)
    for s in range(4):
        for k in range(3):
            base = (s * 3 + k) * DH1
            w1[:, base : base + DH1] = W1[s, 128 * k : 128 * (k + 1), :].astype(BF16)
    w2a = np.empty((128, 4 * DH2), BF16)
    w2b = np.empty((32, 4 * DH2), BF16)
    w3 = np.empty((128, 4 * DH3), BF16)
    w4 = np.empty((96, 4), BF16)
    b1a = np.empty((128, 4), F32)
    b1b = np.empty((32, 4), F32)
    b2p = np.empty((128, 4), F32)
    b3p = np.empty((96, 4), F32)
    b4p = np.empty(4, F32)
    for s in range(4):
        w2a[:, s * DH2 : (s + 1) * DH2] = W2[s, :128, :].astype(BF16)
        w2b[:, s * DH2 : (s + 1) * DH2] = W2[s, 128:, :].astype(BF16)
        w3[:, s * DH3 : (s + 1) * DH3] = W3[s].astype(BF16)
        w4[:, s] = W4[s, :, 0].astype(BF16)
        b1a[:, s] = b1[s, :128]
        b1b[:, s] = b1[s, 128:]
        # fold the celu+1 shift of the previous layer into this layer's bias
        b2p[:, s] = b2[s] - W2[s].sum(axis=0)
        b3p[:, s] = b3[s] - W3[s].sum(axis=0)
        b4p[s] = b4[s, 0] - W4[s, :, 0].sum()
    return dict(w1=w1, w2a=w2a, w2b=w2b, w3=w3, w4=w4,
                b1a=b1a, b1b=b1b, b2=b2p, b3=b3p), b4p


def _route(species, aev):
    """Sort atoms by species, pad per species to 8*TILE multiples, deal to
    cores. Returns (x_per_core [8,128,3,ncore] bf16, slotmap [8,ncore] int64,
    seg_tiles [4])."""
    n = species.size
    sp = species.reshape(-1)
    x = aev.reshape(n, D_AEV)
    seg_tiles = []
    per_core_ids = []
    for s in range(N_SPECIES):
        ids = np.nonzero(sp == s)[0]
        t = max(1, int(np.ceil(len(ids) / (N_CORES * TILE))))
        seg_tiles.append(t)
        padded = np.full(N_CORES * t * TILE, -1, np.int64)
        padded[: len(ids)] = ids
        per_core_ids.append(padded.reshape(N_CORES, t * TILE))
    slotmap = np.concatenate(per_core_ids, axis=1)  # [8, ncore]
    ncore = slotmap.shape[1]

    x_bf = x.astype(BF16)
    x_cores = np.zeros((N_CORES, ncore, D_AEV), BF16)
    for i in range(N_CORES):
        valid = slotmap[i] >= 0
        x_cores[i, valid] = x_bf[slotmap[i][valid]]
    # device layout: [128, 3, ncore] with feature f = c*128 + p
    xT = np.ascontiguousarray(
        x_cores.reshape(N_CORES, ncore, 3, 128).transpose(0, 3, 2, 1)
    )
    return xT, slotmap, seg_tiles


_GRAPH_CACHE = {}


def kernel(species, aev, W1, b1, W2, b2, W3, b3, W4, b4):
    species = np.asarray(species)
    aev = np.asarray(aev, F32)
    B, A = species.shape

    xT, slotmap, seg_tiles = _route(species, aev)
    wmap, b4p = _prep_weights(
        np.asarray(W1, F32), np.asarray(b1, F32), np.asarray(W2, F32),
        np.asarray(b2, F32), np.asarray(W3, F32), np.asarray(b3, F32),
        np.asarray(W4, F32), np.asarray(b4, F32),
    )

    key = tuple(seg_tiles)
    if key not in _GRAPH_CACHE:
        _GRAPH_CACHE[key] = build_graph(seg_tiles)
    nc = _GRAPH_CACHE[key]

    in_maps = [{"x": xT[i], **wmap} for i in range(N_CORES)]
    res = run_bass_kernel_spmd(
        nc,
        in_maps,
        core_ids=list(range(N_CORES)),
        trace=bool(os.environ.get("ANI_TRACE")),
    )
    kernel.last_result = res
    if res.exec_time_ns is not None:
        print(f"HW exec time: {res.exec_time_ns} ns")

    n = B * A
    y_atoms = np.zeros(n, F32)
    for i in range(N_CORES):
        valid = slotmap[i] >= 0
        y_atoms[slotmap[i][valid]] = res.results[i]["out"][0][valid]
    y_atoms += b4p[species.reshape(-1)]
    return y_atoms.reshape(B, A).sum(axis=-1).astype(F32)


# revision 7
# speedup vs baseline: 1.1499x; 1.1499x over previous
"""ANI-style per-species MLP (MoE hard routing) on 8 TRN2 NeuronCores.

Strategy:
  - Host: flatten atoms, sort by species, pad each species bucket to a
    multiple of 8*TILE, and deal equal per-species segments to each core.
    Every core therefore runs the SAME graph (SPMD) over its own atoms.
  - Device (per core): for each 512-atom tile of a species segment, run the
    4-layer MLP (384->160->128->96->1, CELU) with that species' weights only
    (4x less work than the dense reference). Matmuls in bf16 (fp32 PSUM
    accumulate), CELU via one ScalarE Exp pass + one fused custom DVE op:
        celu(v) = relu(v) + (min(exp(v), 1) - 1)       [v = z + b]
  - Host: scatter per-atom energies back, add the L4 bias, per-molecule sum.
"""

import os
from contextlib import ExitStack

import numpy as np
import ml_dtypes

import concourse.bacc as bacc
import concourse.mybir as mybir
import concourse.tile as tile
from concourse.bass_utils import run_bass_kernel_spmd

BF16 = ml_dtypes.bfloat16
F32 = np.float32

N_CORES = 8
TILE = 512
D_AEV = 384
DH1, DH2, DH3 = 160, 128, 96
N_SPECIES = 4

_ACT = mybir.ActivationFunctionType


# --------------------------------------------------------------------------- #
# Fused CELU custom DVE op: out = (min(in0, 1) - 1) + relu(in1 + s0)
#   in0 = exp(z + b) (SBUF f32, from ScalarE), in1 = z (PSUM f32), s0 = b.
# Result equals celu(z + b) exactly.
# --------------------------------------------------------------------------- #
def _register_celu_op():
    import concourse.dve_ops as dve_ops
    from concourse.dve_spec import Spec, Src0, Src1, C0, One, relu, minn, lower
    from concourse.dve_uop import DveOpSpec

    name = "CELU1_ANT"
    for op in dve_ops.OPS:
        if op.name == name:
            return op
    spec = Spec(
        body=(minn(Src0, One) - One) + relu(Src1 + C0),
        reference=lambda in0, in1, s0, s1, imm2: (np.minimum(in0, 1.0) - 1.0)
        + np.maximum(in1 + s0, 0.0),
    )
    row = dve_ops._CUSTOM_DVE_ROW_BASE + len(dve_ops.OPS)
    assert row < 0x20, "custom DVE row field overflow"
    shas = {}
    for ver in ("v3", "v4"):
        d = DveOpSpec(name=name, opcode=row, uops=lower(spec, ver=ver), rd1_en=True)
        shas[ver] = d.sha(ver)
    op = dve_ops.DveOp(name, spec, False, shas)
    dve_ops.OPS.append(op)
    dve_ops.CUSTOM_DVE_SPECS[name] = spec
    dve_ops._SUB_OPCODE_FOR_NAME[name] = row
    return op


# --------------------------------------------------------------------------- #
# Graph builder (one core's SPMD program).
# seg_tiles: number of 512-atom tiles per species segment. ncore = 512*sum.
# --------------------------------------------------------------------------- #
def build_graph(seg_tiles):
    celu_op = _register_celu_op()
    dt = mybir.dt
    ncore = TILE * int(sum(seg_tiles))

    nc = bacc.Bacc("TRN2", target_bir_lowering=False, debug=False)

    x_ext = nc.dram_tensor("x", [128, 3, ncore], dt.bfloat16, kind="ExternalInput")
    w1_ext = nc.dram_tensor("w1", [128, 12 * DH1], dt.bfloat16, kind="ExternalInput")
    w2a_ext = nc.dram_tensor("w2a", [128, 4 * DH2], dt.bfloat16, kind="ExternalInput")
    w2b_ext = nc.dram_tensor("w2b", [32, 4 * DH2], dt.bfloat16, kind="ExternalInput")
    w3_ext = nc.dram_tensor("w3", [128, 4 * DH3], dt.bfloat16, kind="ExternalInput")
    w4_ext = nc.dram_tensor("w4", [96, 4], dt.bfloat16, kind="ExternalInput")
    b1a_ext = nc.dram_tensor("b1a", [128, 4], dt.float32, kind="ExternalInput")
    b1b_ext = nc.dram_tensor("b1b", [32, 4], dt.float32, kind="ExternalInput")
    b2_ext = nc.dram_tensor("b2", [128, 4], dt.float32, kind="ExternalInput")
    b3_ext = nc.dram_tensor("b3", [96, 4], dt.float32, kind="ExternalInput")
    out_ext = nc.dram_tensor("out", [1, ncore], dt.float32, kind="ExternalOutput")

    with tile.TileContext(nc) as tc, ExitStack() as ctx:
        wpool = ctx.enter_context(tc.tile_pool(name="w", bufs=1))
        xpool = ctx.enter_context(tc.tile_pool(name="x", bufs=4))
        epool = ctx.enter_context(tc.tile_pool(name="e", bufs=3))
        spool = ctx.enter_context(tc.tile_pool(name="s", bufs=3))
        p1a = ctx.enter_context(tc.tile_pool(name="p1a", bufs=2, space="PSUM"))
        p1b = ctx.enter_context(tc.tile_pool(name="p1b", bufs=1, space="PSUM"))
        p2 = ctx.enter_context(tc.tile_pool(name="p2", bufs=2, space="PSUM"))
        p3 = ctx.enter_context(tc.tile_pool(name="p3", bufs=1, space="PSUM"))
        p4 = ctx.enter_context(tc.tile_pool(name="p4", bufs=2, space="PSUM"))

        # --- load weights/biases once ---
        w1_sb = wpool.tile([128, 12 * DH1], dt.bfloat16)
        w2a_sb = wpool.tile([128, 4 * DH2], dt.bfloat16)
        w2b_sb = wpool.tile([32, 4 * DH2], dt.bfloat16)
        w3_sb = wpool.tile([128, 4 * DH3], dt.bfloat16)
        w4_sb = wpool.tile([96, 4], dt.bfloat16)
        b1a_sb = wpool.tile([128, 4], dt.float32)
        b1b_sb = wpool.tile([32, 4], dt.float32)
        b2_sb = wpool.tile([128, 4], dt.float32)
        b3_sb = wpool.tile([96, 4], dt.float32)
        for sb, ext in [
            (w1_sb, w1_ext), (w2a_sb, w2a_ext), (w2b_sb, w2b_ext),
            (w3_sb, w3_ext), (w4_sb, w4_ext), (b1a_sb, b1a_ext),
            (b1b_sb, b1b_ext), (b2_sb, b2_ext), (b3_sb, b3_ext),
        ]:
            nc.sync.dma_start(sb[:], ext[:])

        def emit_tile(s, g0):
            xt = xpool.tile([128, 3, TILE], dt.bfloat16)
            nc.sync.dma_start(xt[:], x_ext[:, :, g0 : g0 + TILE])

            # ---- L1: 384 -> 160 (main 128 + rem 32) ----
            z1a = p1a.tile([128, TILE], dt.float32)
            z1b = p1b.tile([32, TILE], dt.float32)
            for k in range(3):
                base = (s * 3 + k) * DH1
                nc.tensor.matmul(
                    z1a[:], w1_sb[:, base : base + 128], xt[:, k, :],
                    start=(k == 0), stop=(k == 2),
                )
            for k in range(3):
                base = (s * 3 + k) * DH1
                nc.tensor.matmul(
                    z1b[:], w1_sb[:, base + 128 : base + DH1], xt[:, k, :],
                    start=(k == 0), stop=(k == 2),
                )
            e1a = epool.tile([128, TILE], dt.float32, tag="e1a")
            nc.scalar.activation(e1a[:], z1a[:], _ACT.Exp, bias=b1a_sb[:, s : s + 1])
            s1a = spool.tile([128, TILE], dt.bfloat16, tag="s1a")
            nc.vector._custom_dve(
                celu_op, out=s1a[:], in0=e1a[:], in1=z1a[:], s0=b1a_sb[:, s : s + 1]
            )
            e1b = epool.tile([32, TILE], dt.float32, tag="e1b")
            nc.scalar.activation(e1b[:], z1b[:], _ACT.Exp, bias=b1b_sb[:, s : s + 1])
            s1b = spool.tile([32, TILE], dt.bfloat16, tag="s1b")
            nc.vector._custom_dve(
                celu_op, out=s1b[:], in0=e1b[:], in1=z1b[:], s0=b1b_sb[:, s : s + 1]
            )

            # ---- L2: 160 -> 128 ----
            z2 = p2.tile([128, TILE], dt.float32)
            nc.tensor.matmul(
                z2[:], w2a_sb[:, s * DH2 : (s + 1) * DH2], s1a[:],
                start=True, stop=False,
            )
            nc.tensor.matmul(
                z2[:], w2b_sb[:, s * DH2 : (s + 1) * DH2], s1b[:],
                start=False, stop=True,
            )
            e2 = epool.tile([128, TILE], dt.float32, tag="e2")
            nc.scalar.activation(e2[:], z2[:], _ACT.Exp, bias=b2_sb[:, s : s + 1])
            s2 = spool.tile([128, TILE], dt.bfloat16, tag="s2")
            nc.vector._custom_dve(
                celu_op, out=s2[:], in0=e2[:], in1=z2[:], s0=b2_sb[:, s : s + 1]
            )

            # ---- L3: 128 -> 96 ----
            z3 = p3.tile([96, TILE], dt.float32)
            nc.tensor.matmul(z3[:], w3_sb[:, s * DH3 : (s + 1) * DH3], s2[:])
            e3 = epool.tile([96, TILE], dt.float32, tag="e3")
            nc.scalar.activation(e3[:], z3[:], _ACT.Exp, bias=b3_sb[:, s : s + 1])
            s3 = spool.tile([96, TILE], dt.bfloat16, tag="s3")
            nc.vector._custom_dve(
                celu_op, out=s3[:], in0=e3[:], in1=z3[:], s0=b3_sb[:, s : s + 1]
            )

            # ---- L4: 96 -> 1 (bias added on host) ----
            z4 = p4.tile([1, TILE], dt.float32)
            nc.tensor.matmul(z4[:], w4_sb[:, s : s + 1], s3[:])
            en = spool.tile([1, TILE], dt.float32, tag="en")
            nc.scalar.copy(en[:], z4[:])
            nc.sync.dma_start(out_ext[:, g0 : g0 + TILE], en[:])

        g0 = 0
        for s in range(N_SPECIES):
            for _ in range(int(seg_tiles[s])):
                emit_tile(s, g0)
                g0 += TILE

    nc.compile()
    return nc


# --------------------------------------------------------------------------- #
# Host-side input prep / output unpack.
# --------------------------------------------------------------------------- #
def _prep_weights(W1, b1, W2, b2, W3, b3, W4, b4):
    # w1: [128, 12*DH1], column block (s*3+k) holds W1[s][128k:128k+128, :]
    w1 = np.empty((128, 12 * DH1), BF16)
    for s in range(4):
        for k in range(3):
            base = (s * 3 + k) * DH1
            w1[:, base : base + DH1] = W1[s, 128 * k : 128 * (k + 1), :].astype(BF16)
    w2a = np.empty((128, 4 * DH2), BF16)
    w2b = np.empty((32, 4 * DH2), BF16)
    w3 = np.empty((128, 4 * DH3), BF16)
    w4 = np.empty((96, 4), BF16)
    b1a = np.empty((128, 4), F32)
    b1b = np.empty((32, 4), F32)
    b2p = np.empty((128, 4), F32)
    b3p = np.empty((96, 4), F32)
    b4p = np.empty(4, F32)
    for s in range(4):
        w2a[:, s * DH2 : (s + 1) * DH2] = W2[s, :128, :].astype(BF16)
        w2b[:, s * DH2 : (s + 1) * DH2] = W2[s, 128:, :].astype(BF16)
        w3[:, s * DH3 : (s + 1) * DH3] = W3[s].astype(BF16)
        w4[:, s] = W4[s, :, 0].astype(BF16)
        b1a[:, s] = b1[s, :128]
        b1b[:, s] = b1[s, 128:]
        b2p[:, s] = b2[s]
        b3p[:, s] = b3[s]
        b4p[s] = b4[s, 0]
    return dict(w1=w1, w2a=w2a, w2b=w2b, w3=w3, w4=w4,
                b1a=b1a, b1b=b1b, b2=b2p, b3=b3p), b4p


def _route(species, aev):
    """Sort atoms by species, pad per species to 8*TILE multiples, deal to
    cores. Returns (x_per_core [8,128,3,ncore] bf16, slotmap [8,ncore] int64,
    seg_tiles [4])."""
    n = species.size
    sp = species.reshape(-1)
    x = aev.reshape(n, D_AEV)
    seg_tiles = []
    per_core_ids = []
    for s in range(N_SPECIES):
        ids = np.nonzero(sp == s)[0]
        t = max(1, int(np.ceil(len(ids) / (N_CORES * TILE))))
        seg_tiles.append(t)
        padded = np.full(N_CORES * t * TILE, -1, np.int64)
        padded[: len(ids)] = ids
        per_core_ids.append(padded.reshape(N_CORES, t * TILE))
    slotmap = np.concatenate(per_core_ids, axis=1)  # [8, ncore]
    ncore = slotmap.shape[1]

    x_bf = x.astype(BF16)
    x_cores = np.zeros((N_CORES, ncore, D_AEV), BF16)
    for i in range(N_CORES):
        valid = slotmap[i] >= 0
        x_cores[i, valid] = x_bf[slotmap[i][valid]]
    # device layout: [128, 3, ncore] with feature f = c*128 + p
    xT = np.ascontiguousarray(
        x_cores.reshape(N_CORES, ncore, 3, 128).transpose(0, 3, 2, 1)
    )
    return xT, slotmap, seg_tiles


_GRAPH_CACHE = {}


def kernel(species, aev, W1, b1, W2, b2, W3, b3, W4, b4):
    species = np.asarray(species)
    aev = np.asarray(aev, F32)
    B, A = species.shape

    xT, slotmap, seg_tiles = _route(species, aev)
    wmap, b4p = _prep_weights(
        np.asarray(W1, F32), np.asarray(b1, F32), np.asarray(W2, F32),
        np.asarray(b2, F32), np.asarray(W3, F32), np.asarray(b3, F32),
        np.asarray(W4, F32), np.asarray(b4, F32),
    )

    key = tuple(seg_tiles)
    if key not in _GRAPH_CACHE:
        _GRAPH_CACHE[key] = build_graph(seg_tiles)
    nc = _GRAPH_CACHE[key]

    in_maps = [{"x": xT[i], **wmap} for i in range(N_CORES)]
    res = run_bass_kernel_spmd(
        nc,
        in_maps,
        core_ids=list(range(N_CORES)),
        trace=bool(os.environ.get("ANI_TRACE")),
    )
    kernel.last_result = res
    if res.exec_time_ns is not None:
        print(f"HW exec time: {res.exec_time_ns} ns")

    n = B * A
    y_atoms = np.zeros(n, F32)
    for i in range(N_CORES):
        valid = slotmap[i] >= 0
        y_atoms[slotmap[i][valid]] = res.results[i]["out"][0][valid]
    y_atoms += b4p[species.reshape(-1)]
    return y_atoms.reshape(B, A).sum(axis=-1).astype(F32)


# revision 9
# speedup vs baseline: 1.2973x; 1.1282x over previous
"""ANI-style per-species MLP (MoE hard routing) on 8 TRN2 NeuronCores.

Strategy:
  - Host: flatten atoms, sort by species, pad each species bucket to a
    multiple of 8*TILE, and deal equal per-species segments to each core.
    Every core therefore runs the SAME graph (SPMD) over its own atoms.
  - Device (per core): for each 512-atom tile of a species segment, run the
    4-layer MLP (384->160->128->96->1, CELU) with that species' weights only
    (4x less work than the dense reference). Matmuls in bf16 (fp32 PSUM
    accumulate), CELU via one ScalarE Exp pass + one fused custom DVE op:
        celu(v) = relu(v) + (min(exp(v), 1) - 1)       [v = z + b]
  - Host: scatter per-atom energies back, add the L4 bias, per-molecule sum.
"""

import os
from contextlib import ExitStack

import numpy as np
import ml_dtypes

import concourse.bacc as bacc
import concourse.mybir as mybir
import concourse.tile as tile
from concourse.bass_utils import run_bass_kernel_spmd

BF16 = ml_dtypes.bfloat16
F32 = np.float32

N_CORES = 8
TILE = 512
D_AEV = 384
DH1, DH2, DH3 = 160, 128, 96
N_SPECIES = 4

_ACT = mybir.ActivationFunctionType


# --------------------------------------------------------------------------- #
# Fused CELU custom DVE op: out = (min(in0, 1) - 1) + relu(in1 + s0)
#   in0 = exp(z + b) (SBUF f32, from ScalarE), in1 = z (PSUM f32), s0 = b.
# Result equals celu(z + b) exactly.
# --------------------------------------------------------------------------- #
def _register_celu_op():
    import concourse.dve_ops as dve_ops
    from concourse.dve_spec import Spec, Src0, Src1, C0, One, relu, minn, lower
    from concourse.dve_uop import DveOpSpec

    name = "CELU1_ANT"
    for op in dve_ops.OPS:
        if op.name == name:
            return op
    spec = Spec(
        body=(minn(Src0, One) - One) + relu(Src1 + C0),
        reference=lambda in0, in1, s0, s1, imm2: (np.minimum(in0, 1.0) - 1.0)
        + np.maximum(in1 + s0, 0.0),
    )
    row = dve_ops._CUSTOM_DVE_ROW_BASE + len(dve_ops.OPS)
    assert row < 0x20, "custom DVE row field overflow"
    shas = {}
    for ver in ("v3", "v4"):
        d = DveOpSpec(name=name, opcode=row, uops=lower(spec, ver=ver), rd1_en=True)
        shas[ver] = d.sha(ver)
    op = dve_ops.DveOp(name, spec, False, shas)
    dve_ops.OPS.append(op)
    dve_ops.CUSTOM_DVE_SPECS[name] = spec
    dve_ops._SUB_OPCODE_FOR_NAME[name] = row
    return op


# --------------------------------------------------------------------------- #
# Graph builder (one core's SPMD program).
# seg_tiles: number of 512-atom tiles per species segment. ncore = 512*sum.
# --------------------------------------------------------------------------- #
def build_graph(seg_tiles):
    celu_op = _register_celu_op()
    dt = mybir.dt
    ncore = TILE * int(sum(seg_tiles))

    nc = bacc.Bacc("TRN2", target_bir_lowering=False, debug=False)

    x_ext = nc.dram_tensor("x", [128, 3, ncore], dt.bfloat16, kind="ExternalInput")
    w1_ext = nc.dram_tensor("w1", [128, 12 * DH1], dt.bfloat16, kind="ExternalInput")
    w2a_ext = nc.dram_tensor("w2a", [128, 4 * DH2], dt.bfloat16, kind="ExternalInput")
    w2b_ext = nc.dram_tensor("w2b", [32, 4 * DH2], dt.bfloat16, kind="ExternalInput")
    w3_ext = nc.dram_tensor("w3", [128, 4 * DH3], dt.bfloat16, kind="ExternalInput")
    w4_ext = nc.dram_tensor("w4", [96, 4], dt.bfloat16, kind="ExternalInput")
    b1a_ext = nc.dram_tensor("b1a", [128, 4], dt.float32, kind="ExternalInput")
    b1b_ext = nc.dram_tensor("b1b", [32, 4], dt.float32, kind="ExternalInput")
    b2_ext = nc.dram_tensor("b2", [128, 4], dt.float32, kind="ExternalInput")
    b3_ext = nc.dram_tensor("b3", [96, 4], dt.float32, kind="ExternalInput")
    out_ext = nc.dram_tensor("out", [1, ncore], dt.float32, kind="ExternalOutput")

    with tile.TileContext(nc) as tc, ExitStack() as ctx:
        wpool = ctx.enter_context(tc.tile_pool(name="w", bufs=1))
        xpool = ctx.enter_context(tc.tile_pool(name="x", bufs=4))
        epool = ctx.enter_context(tc.tile_pool(name="e", bufs=3))
        spool = ctx.enter_context(tc.tile_pool(name="s", bufs=3))
        p1a = ctx.enter_context(tc.tile_pool(name="p1a", bufs=2, space="PSUM"))
        p1b = ctx.enter_context(tc.tile_pool(name="p1b", bufs=1, space="PSUM"))
        p2 = ctx.enter_context(tc.tile_pool(name="p2", bufs=2, space="PSUM"))
        p3 = ctx.enter_context(tc.tile_pool(name="p3", bufs=1, space="PSUM"))
        p4 = ctx.enter_context(tc.tile_pool(name="p4", bufs=2, space="PSUM"))

        # --- load weights/biases once ---
        w1_sb = wpool.tile([128, 12 * DH1], dt.bfloat16)
        w2a_sb = wpool.tile([128, 4 * DH2], dt.bfloat16)
        w2b_sb = wpool.tile([32, 4 * DH2], dt.bfloat16)
        w3_sb = wpool.tile([128, 4 * DH3], dt.bfloat16)
        w4_sb = wpool.tile([96, 4], dt.bfloat16)
        b1a_sb = wpool.tile([128, 4], dt.float32)
        b1b_sb = wpool.tile([32, 4], dt.float32)
        b2_sb = wpool.tile([128, 4], dt.float32)
        b3_sb = wpool.tile([96, 4], dt.float32)
        for sb, ext in [
            (w1_sb, w1_ext), (w2a_sb, w2a_ext), (w2b_sb, w2b_ext),
            (w3_sb, w3_ext), (w4_sb, w4_ext), (b1a_sb, b1a_ext),
            (b1b_sb, b1b_ext), (b2_sb, b2_ext), (b3_sb, b3_ext),
        ]:
            nc.sync.dma_start(sb[:], ext[:])

        def celu(z, bias, shape):
            e = epool.tile(shape, dt.float32, tag=f"e{shape[0]}")
            nc.scalar.activation(e[:], z[:], _ACT.Exp, bias=bias)
            sx = spool.tile(shape, dt.bfloat16, tag=f"s{shape[0]}")
            nc.vector._custom_dve(celu_op, out=sx[:], in0=e[:], in1=z[:], s0=bias)
            return sx

        def stage1(st):
            s = st["s"]
            xt = xpool.tile([128, 3, TILE], dt.bfloat16)
            nc.sync.dma_start(xt[:], x_ext[:, :, st["g0"] : st["g0"] + TILE])
            z1a = p1a.tile([128, TILE], dt.float32)
            z1b = p1b.tile([32, TILE], dt.float32)
            for k in range(3):
                base = (s * 3 + k) * DH1
                nc.tensor.matmul(
                    z1a[:], w1_sb[:, base : base + 128], xt[:, k, :],
                    start=(k == 0), stop=(k == 2),
                )
            for k in range(3):
                base = (s * 3 + k) * DH1
                nc.tensor.matmul(
                    z1b[:], w1_sb[:, base + 128 : base + DH1], xt[:, k, :],
                    start=(k == 0), stop=(k == 2),
                )
            st["s1a"] = celu(z1a, b1a_sb[:, s : s + 1], [128, TILE])
            st["s1b"] = celu(z1b, b1b_sb[0:32, s : s + 1], [32, TILE])

        def stage2(st):
            s = st["s"]
            z2 = p2.tile([128, TILE], dt.float32)
            nc.tensor.matmul(
                z2[:], w2a_sb[:, s * DH2 : (s + 1) * DH2], st["s1a"][:],
                start=True, stop=False,
            )
            nc.tensor.matmul(
                z2[:], w2b_sb[:, s * DH2 : (s + 1) * DH2], st["s1b"][:],
                start=False, stop=True,
            )
            st["s2"] = celu(z2, b2_sb[:, s : s + 1], [128, TILE])

        def stage3(st):
            s = st["s"]
            z3 = p3.tile([96, TILE], dt.float32)
            nc.tensor.matmul(z3[:], w3_sb[:, s * DH3 : (s + 1) * DH3], st["s2"][:])
            st["s3"] = celu(z3, b3_sb[0:96, s : s + 1], [96, TILE])

        def stage4(st):
            s = st["s"]
            z4 = p4.tile([1, TILE], dt.float32)
            nc.tensor.matmul(z4[:], w4_sb[:, s : s + 1], st["s3"][:])
            en = spool.tile([1, TILE], dt.float32, tag="en")
            nc.scalar.copy(en[:], z4[:])
            nc.sync.dma_start(out_ext[:, st["g0"] : st["g0"] + TILE], en[:])

        tiles = []
        g0 = 0
        for s in range(N_SPECIES):
            for _ in range(int(seg_tiles[s])):
                tiles.append({"s": s, "g0": g0})
                g0 += TILE

        # software-pipeline skew: tile t's layer l is emitted at step t + l,
        # so every engine always has independent work from adjacent tiles.
        n = len(tiles)
        for step in range(n + 3):
            if step < n:
                stage1(tiles[step])
            if 0 <= step - 1 < n:
                stage2(tiles[step - 1])
            if 0 <= step - 2 < n:
                stage3(tiles[step - 2])
            if 0 <= step - 3 < n:
                stage4(tiles[step - 3])

    nc.compile()
    return nc


# --------------------------------------------------------------------------- #
# Host-side input prep / output unpack.
# --------------------------------------------------------------------------- #
def _prep_weights(W1, b1, W2, b2, W3, b3, W4, b4):
    # w1: [128, 12*DH1], column block (s*3+k) holds W1[s][128k:128k+128, :]
    w1 = np.empty((128, 12 * DH1), BF16)
    for s in range(4):
        for k in range(3):
            base = (s * 3 + k) * DH1
            w1[:, base : base + DH1] = W1[s, 128 * k : 128 * (k + 1), :].astype(BF16)
    w2a = np.empty((128, 4 * DH2), BF16)
    w2b = np.empty((32, 4 * DH2), BF16)
    w3 = np.empty((128, 4 * DH3), BF16)
    w4 = np.empty((96, 4), BF16)
    b1a = np.empty((128, 4), F32)
    b1b = np.empty((32, 4), F32)
    b2p = np.empty((128, 4), F32)
    b3p = np.empty((96, 4), F32)
    b4p = np.empty(4, F32)
    for s in range(4):
        w2a[:, s * DH2 : (s + 1) * DH2] = W2[s, :128, :].astype(BF16)
        w2b[:, s * DH2 : (s + 1) * DH2] = W2[s, 128:, :].astype(BF16)
        w3[:, s * DH3 : (s + 1) * DH3] = W3[s].astype(BF16)
        w4[:, s] = W4[s, :, 0].astype(BF16)
        b1a[:, s] = b1[s, :128]
        b1b[:, s] = b1[s, 128:]
        b2p[:, s] = b2[s]
        b3p[:, s] = b3[s]
        b4p[s] = b4[s, 0]
    return dict(w1=w1, w2a=w2a, w2b=w2b, w3=w3, w4=w4,
                b1a=b1a, b1b=b1b, b2=b2p, b3=b3p), b4p


def _route(species, aev):
    """Sort atoms by species, pad per species to 8*TILE multiples, deal to
    cores. Returns (x_per_core [8,128,3,ncore] bf16, slotmap [8,ncore] int64,
    seg_tiles [4])."""
    n = species.size
    sp = species.reshape(-1)
    x = aev.reshape(n, D_AEV)
    seg_tiles = []
    per_core_ids = []
    for s in range(N_SPECIES):
        ids = np.nonzero(sp == s)[0]
        t = max(1, int(np.ceil(len(ids) / (N_CORES * TILE))))
        seg_tiles.append(t)
        padded = np.full(N_CORES * t * TILE, -1, np.int64)
        padded[: len(ids)] = ids
        per_core_ids.append(padded.reshape(N_CORES, t * TILE))
    slotmap = np.concatenate(per_core_ids, axis=1)  # [8, ncore]
    ncore = slotmap.shape[1]

    x_bf = x.astype(BF16)
    x_cores = np.zeros((N_CORES, ncore, D_AEV), BF16)
    for i in range(N_CORES):
        valid = slotmap[i] >= 0
        x_cores[i, valid] = x_bf[slotmap[i][valid]]
    # device layout: [128, 3, ncore] with feature f = c*128 + p
    xT = np.ascontiguousarray(
        x_cores.reshape(N_CORES, ncore, 3, 128).transpose(0, 3, 2, 1)
    )
    return xT, slotmap, seg_tiles


_GRAPH_CACHE = {}


def kernel(species, aev, W1, b1, W2, b2, W3, b3, W4, b4):
    species = np.asarray(species)
    aev = np.asarray(aev, F32)
    B, A = species.shape

    xT, slotmap, seg_tiles = _route(species, aev)
    wmap, b4p = _prep_weights(
        np.asarray(W1, F32), np.asarray(b1, F32), np.asarray(W2, F32),
        np.asarray(b2, F32), np.asarray(W3, F32), np.asarray(b3, F32),
        np.asarray(W4, F32), np.asarray(b4, F32),
    )

    key = tuple(seg_tiles)
    if key not in _GRAPH_CACHE:
        _GRAPH_CACHE[key] = build_graph(seg_tiles)
    nc = _GRAPH_CACHE[key]

    in_maps = [{"x": xT[i], **wmap} for i in range(N_CORES)]
    res = run_bass_kernel_spmd(
        nc,
        in_maps,
        core_ids=list(range(N_CORES)),
        trace=bool(os.environ.get("ANI_TRACE")),
    )
    kernel.last_result = res
    if res.exec_time_ns is not None:
        print(f"HW exec time: {res.exec_time_ns} ns")

    n = B * A
    y_atoms = np.zeros(n, F32)
    for i in range(N_CORES):
        valid = slotmap[i] >= 0
        y_atoms[slotmap[i][valid]] = res.results[i]["out"][0][valid]
    y_atoms += b4p[species.reshape(-1)]
    return y_atoms.reshape(B, A).sum(axis=-1).astype(F32)
